# revision 12
# baseline (speedup 1.0000x reference)
"""Distributed multi-head attention (B=2, S=2048, D=1024, H=16) on 8 NeuronCores.

Sharding: tokens are flattened to 4096 rows and split 512 rows/core (core c
owns batch c//4, sequence block c%4).  Each core projects Q/K/V for its own
rows (QK-layernorm is purely local since every core holds full d_model rows),
then K^T and V are AllGathered within each 4-core batch group so every core
holds the full-sequence K/V for its batch.  Attention and the output
projection then run fully locally for the core's 512 query rows (all heads),
so the final output needs no collective — the host just concatenates row
blocks.  All matmuls run in float32r (full-rate PE mode).
"""

import sys

sys.path.insert(0, "/opt/trn_rl_repo")

import numpy as np
import concourse.bass as bass
import concourse.tile as tile
from concourse import bacc, mybir
from concourse.bass_utils import run_bass_kernel_spmd
from concourse.masks import make_identity

F32 = mybir.dt.float32
F32R = mybir.dt.float32r

N_CORES = 8
B, S, D = 2, 2048, 1024
H, DK = 16, 64
ROWS = 512  # token rows per core
LN_EPS = 1e-5
SCALE = 0.125  # 1/sqrt(DK)

KT_ELEMS = D * ROWS  # 524288, K^T block in kv_in
KV_ELEMS = 2 * KT_ELEMS  # K^T + V per core

REPLICA_GROUPS = [[0, 1, 2, 3], [4, 5, 6, 7]]


def build_kernel(repeat=None, mock_collective=False, skip_kv_loads=False):
    nc = bacc.Bacc("TRN2", target_bir_lowering=False, debug=False, num_devices=N_CORES)

    # Inputs (host pre-transposed): x^T [D, ROWS] slices and W^T [D, D].
    xq = nc.declare_dram_parameter("xq", [D, ROWS], F32R, isOutput=False)
    xk = nc.declare_dram_parameter("xk", [D, ROWS], F32R, isOutput=False)
    xv = nc.declare_dram_parameter("xv", [D, ROWS], F32R, isOutput=False)
    wq = nc.declare_dram_parameter("wq", [D, D], F32R, isOutput=False)
    wk = nc.declare_dram_parameter("wk", [D, D], F32R, isOutput=False)
    wv = nc.declare_dram_parameter("wv", [D, D], F32R, isOutput=False)
    wo = nc.declare_dram_parameter("wo", [D, D], F32R, isOutput=False)
    gamma = nc.declare_dram_parameter("gamma", [D], F32, isOutput=False)
    beta = nc.declare_dram_parameter("beta", [D], F32, isOutput=False)
    out_ext = nc.declare_dram_parameter("out", [ROWS, D], F32, isOutput=True)

    # Collective buffers: [K^T (524288) | V (524288)] flat.
    kv_in = nc.dram_tensor("kv_in", [KV_ELEMS], F32R)
    kv_out = nc.dram_tensor("kv_out", [4 * KV_ELEMS], F32R)

    # DRAM views
    kt_in_v = kv_in[:][0:KT_ELEMS].rearrange("(c p f) -> c p f", p=128, f=ROWS)
    v_in_v = kv_in[:][KT_ELEMS:KV_ELEMS].rearrange("(r p n) -> r p n", p=128, n=D)

    import contextlib

    with tile.TileContext(nc) as tc:
        loop_cm = (
            tc.For_i(
                0,
                repeat,
                1,
                hint_engines=(
                    mybir.EngineType.PE,
                    mybir.EngineType.Activation,
                    mybir.EngineType.DVE,
                    mybir.EngineType.SP,
                    mybir.EngineType.Pool,
                ),
            )
            if repeat
            else contextlib.nullcontext()
        )
        with loop_cm, tc.tile_pool(name="persist", bufs=1) as persist:
            ident = persist.tile([128, 128], F32)
            make_identity(nc, ident)
            eps_t = persist.tile([128, 1], F32)
            nc.vector.memset(eps_t, LN_EPS)
            ones64_f = persist.tile([1, 64], F32)
            nc.vector.memset(ones64_f, 1.0)
            ones64 = persist.tile([1, 64], F32R)
            nc.vector.tensor_copy(ones64, ones64_f)
            ones32 = persist.tile([128, 32], F32)
            nc.vector.memset(ones32, 1.0)
            # gamma/beta per-partition in transposed layout: [128, 8]
            gamma_t = persist.tile([128, 8], F32)
            nc.sync.dma_start(out=gamma_t, in_=gamma[:].rearrange("(c p) -> p c", p=128))
            beta_t = persist.tile([128, 8], F32)
            nc.sync.dma_start(out=beta_t, in_=beta[:].rearrange("(c p) -> p c", p=128))

            qt_sb = persist.tile([128, 8, ROWS], F32R)  # Q^T (post-LN)
            ctx_sb = persist.tile([128, 8, ROWS], F32R)  # context^T
            wo_sb = persist.tile([128, 8, D], F32R)  # Wo^T

            # ---------------- Phase 1: projections + LN + transposes ----------
            def projection(ctx_pools, xin, win, kind):
                """kind: 'q' | 'k' | 'v'."""
                in_pool, w_pool, nat_pool, kt_pool, tmp_pool, ppsum, tpsum = ctx_pools
                xt = in_pool.tile([128, 8, ROWS], F32R, tag="xt")
                xin_v = xin[:].rearrange("(c p) f -> p c f", p=128)
                w_sb = w_pool.tile([128, 8, D], F32R, tag="w")
                win_v = win[:].rearrange("(c p) f -> p c f", p=128)
                for dc in range(8):
                    nc.sync.dma_start(out=xt[:, dc, :], in_=xin_v[:, dc, :])
                    nc.sync.dma_start(out=w_sb[:, dc, :], in_=win_v[:, dc, :])

                kt_sb = None
                if kind == "k":
                    kt_sb = kt_pool.tile([128, 8, ROWS], F32R, tag="kt_stage")

                for rb in range(4):
                    nat = nat_pool.tile([128, D], F32 if kind != "v" else F32R, tag="nat")
                    for ncol in range(2):
                        ps = ppsum.tile([128, 512], F32, tag="proj")
                        for dc in range(8):
                            nc.tensor.matmul(
                                ps,
                                lhsT=xt[:, dc, rb * 128 : (rb + 1) * 128],
                                rhs=w_sb[:, dc, ncol * 512 : (ncol + 1) * 512],
                                start=(dc == 0),
                                stop=(dc == 7),
                            )
                        nc.vector.tensor_copy(nat[:, ncol * 512 : (ncol + 1) * 512], ps)
                    if kind == "v":
                        nc.sync.dma_start(out=v_in_v[rb], in_=nat)
                        continue
                    # LayerNorm over the full row (free dim), gamma/beta applied
                    # post-transpose (where they are per-partition).
                    stats = tmp_pool.tile([128, 2, 6], F32, tag="stats")
                    for i in range(2):
                        nc.vector.bn_stats(
                            out=stats[:, i, :], in_=nat[:, i * 512 : (i + 1) * 512]
                        )
                    mv = tmp_pool.tile([128, 2], F32, tag="mv")
                    nc.vector.bn_aggr(out=mv, in_=stats)
                    rstd = tmp_pool.tile([128, 1], F32, tag="rstd")
                    nc.scalar.activation(
                        out=rstd,
                        in_=mv[:, 1:2],
                        func=mybir.ActivationFunctionType.Sqrt,
                        bias=eps_t,
                        scale=1.0,
                    )
                    nc.vector.reciprocal(out=rstd, in_=rstd)
                    nc.vector.tensor_scalar(
                        out=nat,
                        in0=nat,
                        scalar1=mv[:, 0:1],
                        scalar2=rstd,
                        op0=mybir.AluOpType.subtract,
                        op1=mybir.AluOpType.mult,
                    )
                    # transpose 128x128 blocks into [D, ROWS] layout
                    for dc in range(8):
                        tp = tpsum.tile([128, 128], F32, tag="tp")
                        nc.tensor.transpose(tp, nat[:, dc * 128 : (dc + 1) * 128], ident)
                        dst = qt_sb if kind == "q" else kt_sb
                        nc.vector.tensor_scalar(
                            out=dst[:, dc, rb * 128 : (rb + 1) * 128],
                            in0=tp,
                            scalar1=gamma_t[:, dc : dc + 1],
                            scalar2=beta_t[:, dc : dc + 1],
                            op0=mybir.AluOpType.mult,
                            op1=mybir.AluOpType.add,
                        )
                if kind == "k":
                    for dc in range(8):
                        nc.sync.dma_start(out=kt_in_v[dc], in_=kt_sb[:, dc, :])

            with (
                tc.tile_pool(name="p1_in", bufs=2) as in_pool,
                tc.tile_pool(name="p1_w", bufs=2) as w_pool,
                tc.tile_pool(name="p1_nat", bufs=3) as nat_pool,
                tc.tile_pool(name="p1_kt", bufs=1) as kt_pool,
                tc.tile_pool(name="p1_tmp", bufs=4) as tmp_pool,
                tc.tile_pool(name="ppsum", bufs=2, space="PSUM") as ppsum,
                tc.tile_pool(name="tpsum", bufs=2, space="PSUM") as tpsum,
            ):
                pools = (in_pool, w_pool, nat_pool, kt_pool, tmp_pool, ppsum, tpsum)
                projection(pools, xk, wk, "k")
                projection(pools, xv, wv, "v")
                if mock_collective:
                    for j in range(4):
                        nc.sync.dma_start(
                            out=kv_out[:][j * KV_ELEMS : (j + 1) * KV_ELEMS],
                            in_=kv_in[:],
                        )
                else:
                    nc.gpsimd.collective_compute(
                        "AllGather",
                        mybir.AluOpType.bypass,
                        replica_groups=REPLICA_GROUPS,
                        ins=[kv_in[:]],
                        outs=[kv_out[:]],
                    )
                projection(pools, xq, wq, "q")
                nc.sync.dma_start(
                    out=wo_sb, in_=wo[:].rearrange("(c p) f -> p c f", p=128)
                )

            # ---------------- Phase 3: attention ------------------------------
            kv4 = kv_out[:].rearrange("(j e) -> j e", j=4)  # per-peer blocks

            with (
                tc.tile_pool(name="a_kt", bufs=2) as kt_pool3,
                tc.tile_pool(name="a_v", bufs=2) as v_pool,
                tc.tile_pool(name="a_exp", bufs=2) as e_pool,
                tc.tile_pool(name="a_den", bufs=4) as den_pool,
                tc.tile_pool(name="s_psum", bufs=2, space="PSUM") as s_psum,
                tc.tile_pool(name="c_psum", bufs=1, space="PSUM") as c_psum,
                tc.tile_pool(name="b_psum", bufs=1, space="PSUM") as b_psum,
                tc.tile_pool(name="o_psum", bufs=2, space="PSUM") as o_psum,
                tc.tile_pool(name="out_sb", bufs=2) as out_pool,
            ):
                static_kv = [None, None]
                for hp in range(8):  # head pairs
                    if skip_kv_loads and hp > 0:
                        kt_t, v_t = static_kv
                    else:
                        # K^T for heads (2hp, 2hp+1): [128 dims, 2048 kpos]
                        kt_t = kt_pool3.tile([128, 4, ROWS], F32R, tag="kt")
                        kt_src = (
                            kv4[:, hp * (128 * ROWS) : (hp + 1) * (128 * ROWS)]
                            .rearrange("j (p f) -> p j f", p=128)
                        )
                        nc.sync.dma_start(out=kt_t, in_=kt_src)
                        # V for the pair: [128 kpos, 16 chunks, 2 heads, 65]
                        v_t = v_pool.tile([128, 16, 2, 65], F32R, tag="v")
                        for j in range(4):
                            for hh in range(2):
                                v_src = kv4[j, KT_ELEMS:KV_ELEMS].rearrange(
                                    "(r p n) -> p r n", p=128, n=D
                                )[:, :, 128 * hp + 64 * hh : 128 * hp + 64 * (hh + 1)]
                                nc.scalar.dma_start(
                                    out=v_t[:, j * 4 : (j + 1) * 4, hh, 0:64],
                                    in_=v_src,
                                )
                        nc.vector.tensor_copy(
                            v_t[:, :, :, 64:65].rearrange("p c hh one -> p (c hh one)"),
                            ones32,
                        )
                        static_kv = [kt_t, v_t]

                    for hh in range(2):
                        exp_t = e_pool.tile([128, 16, 512], F32R, tag="exp")
                        for g in range(8):
                            st = s_psum.tile([128, 2, 512], F32, tag="st")
                            for k in range(2):
                                cb = 2 * g + k
                                j, rb = cb // 4, cb % 4
                                nc.tensor.matmul(
                                    st[:, k, :],
                                    lhsT=kt_t[
                                        64 * hh : 64 * (hh + 1),
                                        j,
                                        rb * 128 : (rb + 1) * 128,
                                    ],
                                    rhs=qt_sb[64 * hh : 64 * (hh + 1), hp, :],
                                    start=True,
                                    stop=True,
                                )
                            nc.scalar.activation(
                                out=exp_t[:, 2 * g : 2 * g + 2, :],
                                in_=st,
                                func=mybir.ActivationFunctionType.Exp,
                                scale=SCALE,
                            )
                        ctx_ps = c_psum.tile([128, 512], F32, tag="ctx")
                        for cb in range(16):
                            nc.tensor.matmul(
                                ctx_ps[0:65, :],
                                lhsT=v_t[:, cb, hh, :],
                                rhs=exp_t[:, cb, :],
                                start=(cb == 0),
                                stop=(cb == 15),
                            )
                        den = den_pool.tile([1, 512], F32R, tag="den")
                        with nc.allow_low_precision(
                            reason="reciprocal rounded to f32r to feed PE broadcast"
                        ):
                            nc.vector.reciprocal(out=den, in_=ctx_ps[64:65, :])
                        bc = b_psum.tile([64, 512], F32, tag="bc")
                        nc.tensor.matmul(
                            bc, lhsT=ones64, rhs=den, start=True, stop=True
                        )
                        bc_sb = den_pool.tile([64, 512], F32, tag="bc_sb")
                        nc.vector.tensor_copy(bc_sb, bc)
                        nc.vector.tensor_mul(
                            ctx_sb[64 * hh : 64 * (hh + 1), hp, :],
                            ctx_ps[0:64, :],
                            bc_sb,
                        )

                # ---------------- Output projection ---------------------------
                for qb in range(4):
                    ot = out_pool.tile([128, D], F32, tag="ot")
                    for ncol in range(2):
                        ps = o_psum.tile([128, 512], F32, tag="po")
                        for dc in range(8):
                            nc.tensor.matmul(
                                ps,
                                lhsT=ctx_sb[:, dc, qb * 128 : (qb + 1) * 128],
                                rhs=wo_sb[:, dc, ncol * 512 : (ncol + 1) * 512],
                                start=(dc == 0),
                                stop=(dc == 7),
                            )
                        nc.vector.tensor_copy(ot[:, ncol * 512 : (ncol + 1) * 512], ps)
                    nc.sync.dma_start(out=out_ext[qb * 128 : (qb + 1) * 128, :], in_=ot)

    nc.finalize()
    return nc


_NC_CACHE = None


def _get_nc():
    global _NC_CACHE
    if _NC_CACHE is None:
        _NC_CACHE = build_kernel()
    return _NC_CACHE


def shard_inputs(query, key, value, Wq, Wk, Wv, Wo, ln_gamma, ln_beta):
    q2 = np.ascontiguousarray(query.reshape(B * S, D))
    k2 = np.ascontiguousarray(key.reshape(B * S, D))
    v2 = np.ascontiguousarray(value.reshape(B * S, D))
    wqt = np.ascontiguousarray(Wq.T)
    wkt = np.ascontiguousarray(Wk.T)
    wvt = np.ascontiguousarray(Wv.T)
    wot = np.ascontiguousarray(Wo.T)
    g = np.ascontiguousarray(ln_gamma)
    b = np.ascontiguousarray(ln_beta)
    in_maps = []
    for c in range(N_CORES):
        rows = slice(c * ROWS, (c + 1) * ROWS)
        in_maps.append(
            {
                "xq": np.ascontiguousarray(q2[rows].T),
                "xk": np.ascontiguousarray(k2[rows].T),
                "xv": np.ascontiguousarray(v2[rows].T),
                "wq": wqt,
                "wk": wkt,
                "wv": wvt,
                "wo": wot,
                "gamma": g,
                "beta": b,
            }
        )
    return in_maps


def kernel(query, key, value, Wq, Wk, Wv, Wo, ln_gamma, ln_beta):
    query = np.asarray(query, dtype=np.float32)
    key = np.asarray(key, dtype=np.float32)
    value = np.asarray(value, dtype=np.float32)
    in_maps = shard_inputs(
        query,
        key,
        value,
        np.asarray(Wq, np.float32),
        np.asarray(Wk, np.float32),
        np.asarray(Wv, np.float32),
        np.asarray(Wo, np.float32),
        np.asarray(ln_gamma, np.float32),
        np.asarray(ln_beta, np.float32),
    )
    nc = _get_nc()
    res = run_bass_kernel_spmd(nc, in_maps, core_ids=list(range(N_CORES)))
    out = np.concatenate(
        [res.results[c]["out"] for c in range(N_CORES)], axis=0
    ).reshape(B, S, D)
    return out.astype(np.float32)


# revision 20
# speedup vs baseline: 1.0579x; 1.0579x over previous
"""Distributed multi-head attention (B=2, S=2048, D=1024, H=16) on 8 NeuronCores.

Sharding: tokens are flattened to 4096 rows and split 512 rows/core (core c
owns batch c//4, sequence block c%4).  Each core projects Q/K/V for its own
rows (QK-layernorm is purely local since every core holds full d_model rows),
then K^T and V are AllGathered within each 4-core batch group so every core
holds the full-sequence K/V for its batch.  Attention and the output
projection then run fully locally for the core's 512 query rows (all heads),
so the final output needs no collective — the host just concatenates row
blocks.  All matmuls run in float32r (full-rate PE mode).
"""

import sys

sys.path.insert(0, "/opt/trn_rl_repo")

import numpy as np
import concourse.bass as bass
import concourse.tile as tile
from concourse import bacc, mybir
from concourse.bass_utils import run_bass_kernel_spmd
from concourse.masks import make_identity

F32 = mybir.dt.float32
F32R = mybir.dt.float32r

N_CORES = 8
B, S, D = 2, 2048, 1024
H, DK = 16, 64
ROWS = 512  # token rows per core
LN_EPS = 1e-5
SCALE = 0.125  # 1/sqrt(DK)

KT_ELEMS = D * ROWS  # 524288, K^T block in kv_in
KV_ELEMS = 2 * KT_ELEMS  # K^T + V per core

REPLICA_GROUPS = [[0, 1, 2, 3], [4, 5, 6, 7]]


def build_kernel(repeat=None, mock_collective=False, skip_kv_loads=False, ctx2=False, padk=False, splitag=False, exp1=False):
    nc = bacc.Bacc("TRN2", target_bir_lowering=False, debug=False, num_devices=N_CORES)

    # Inputs (host pre-transposed): x^T [D, ROWS] slices and W^T [D, D].
    xq = nc.declare_dram_parameter("xq", [D, ROWS], F32R, isOutput=False)
    xk = nc.declare_dram_parameter("xk", [D, ROWS], F32R, isOutput=False)
    xv = nc.declare_dram_parameter("xv", [D, ROWS], F32R, isOutput=False)
    wq = nc.declare_dram_parameter("wq", [D, D], F32R, isOutput=False)
    wk = nc.declare_dram_parameter("wk", [D, D], F32R, isOutput=False)
    wv = nc.declare_dram_parameter("wv", [D, D], F32R, isOutput=False)
    wo = nc.declare_dram_parameter("wo", [D, D], F32R, isOutput=False)
    gamma = nc.declare_dram_parameter("gamma", [D], F32, isOutput=False)
    beta = nc.declare_dram_parameter("beta", [D], F32, isOutput=False)
    out_ext = nc.declare_dram_parameter("out", [ROWS, D], F32, isOutput=True)

    # Collective buffers: [K^T (524288) | V (524288)] flat.
    kv_in = nc.dram_tensor("kv_in", [KV_ELEMS], F32R)
    kv_out = nc.dram_tensor("kv_out", [4 * KV_ELEMS], F32R)
    # split-AG variant: separate gathered buffers for K^T and V
    ktg = nc.dram_tensor("ktg", [4 * KT_ELEMS], F32R)
    vg = nc.dram_tensor("vg", [4 * KT_ELEMS], F32R)

    # DRAM views
    kt_in_v = kv_in[:][0:KT_ELEMS].rearrange("(c p f) -> c p f", p=128, f=ROWS)
    v_in_v = kv_in[:][KT_ELEMS:KV_ELEMS].rearrange("(r p n) -> r p n", p=128, n=D)

    import contextlib

    with tile.TileContext(nc) as tc:
        loop_cm = (
            tc.For_i(
                0,
                repeat,
                1,
                hint_engines=(
                    mybir.EngineType.PE,
                    mybir.EngineType.Activation,
                    mybir.EngineType.DVE,
                    mybir.EngineType.SP,
                    mybir.EngineType.Pool,
                ),
            )
            if repeat
            else contextlib.nullcontext()
        )
        with loop_cm, tc.tile_pool(name="persist", bufs=1) as persist:
            ident = persist.tile([128, 128], F32)
            make_identity(nc, ident)
            eps_t = persist.tile([128, 1], F32)
            nc.vector.memset(eps_t, LN_EPS)
            ones64_f = persist.tile([1, 64], F32)
            nc.vector.memset(ones64_f, 1.0)
            ones64 = persist.tile([1, 64], F32R)
            nc.vector.tensor_copy(ones64, ones64_f)
            ones32 = persist.tile([128, 32], F32)
            nc.vector.memset(ones32, 1.0)
            # gamma/beta per-partition in transposed layout: [128, 8]
            gamma_t = persist.tile([128, 8], F32)
            nc.sync.dma_start(out=gamma_t, in_=gamma[:].rearrange("(c p) -> p c", p=128))
            beta_t = persist.tile([128, 8], F32)
            nc.sync.dma_start(out=beta_t, in_=beta[:].rearrange("(c p) -> p c", p=128))

            if padk:
                # per-head Q^T, zero-padded to full 128 contract rows
                qt_sb = persist.tile([128, 16, ROWS], F32R)
                zero64 = persist.tile([64, ROWS], F32)
                nc.vector.memset(zero64, 0.0)
                for h in range(16):
                    lo = 64 if h % 2 == 0 else 0
                    nc.vector.tensor_copy(qt_sb[lo : lo + 64, h, :], zero64)
            else:
                qt_sb = persist.tile([128, 8, ROWS], F32R)  # Q^T (post-LN)
            ctx_sb = persist.tile([128, 8, ROWS], F32R)  # context^T
            wo_sb = persist.tile([128, 8, D], F32R)  # Wo^T

            # ---------------- Phase 1: projections + LN + transposes ----------
            def projection(ctx_pools, xin, win, kind):
                """kind: 'q' | 'k' | 'v'."""
                in_pool, w_pool, nat_pool, kt_pool, tmp_pool, ppsum, tpsum = ctx_pools
                xt = in_pool.tile([128, 8, ROWS], F32R, tag="xt")
                xin_v = xin[:].rearrange("(c p) f -> p c f", p=128)
                w_sb = w_pool.tile([128, 8, D], F32R, tag="w")
                win_v = win[:].rearrange("(c p) f -> p c f", p=128)
                for dc in range(8):
                    nc.sync.dma_start(out=xt[:, dc, :], in_=xin_v[:, dc, :])
                    nc.sync.dma_start(out=w_sb[:, dc, :], in_=win_v[:, dc, :])



                for rb in range(4):
                    nat = nat_pool.tile([128, D], F32 if kind != "v" else F32R, tag="nat")
                    for ncol in range(2):
                        ps = ppsum.tile([128, 512], F32, tag="proj")
                        for dc in range(8):
                            nc.tensor.matmul(
                                ps,
                                lhsT=xt[:, dc, rb * 128 : (rb + 1) * 128],
                                rhs=w_sb[:, dc, ncol * 512 : (ncol + 1) * 512],
                                start=(dc == 0),
                                stop=(dc == 7),
                            )
                        nc.vector.tensor_copy(nat[:, ncol * 512 : (ncol + 1) * 512], ps)
                    if kind == "v":
                        nc.sync.dma_start(out=v_in_v[rb], in_=nat)
                        continue
                    # LayerNorm over the full row (free dim), gamma/beta applied
                    # post-transpose (where they are per-partition).
                    stats = tmp_pool.tile([128, 2, 6], F32, tag="stats")
                    for i in range(2):
                        nc.vector.bn_stats(
                            out=stats[:, i, :], in_=nat[:, i * 512 : (i + 1) * 512]
                        )
                    mv = tmp_pool.tile([128, 2], F32, tag="mv")
                    nc.vector.bn_aggr(out=mv, in_=stats)
                    rstd = tmp_pool.tile([128, 1], F32, tag="rstd")
                    nc.scalar.activation(
                        out=rstd,
                        in_=mv[:, 1:2],
                        func=mybir.ActivationFunctionType.Sqrt,
                        bias=eps_t,
                        scale=1.0,
                    )
                    nc.vector.reciprocal(out=rstd, in_=rstd)
                    nc.vector.tensor_scalar(
                        out=nat,
                        in0=nat,
                        scalar1=mv[:, 0:1],
                        scalar2=rstd,
                        op0=mybir.AluOpType.subtract,
                        op1=mybir.AluOpType.mult,
                    )
                    # transpose 128x128 blocks into [D, ROWS] layout
                    for dc in range(8):
                        tp = tpsum.tile([128, 128], F32, tag="tp")
                        nc.tensor.transpose(tp, nat[:, dc * 128 : (dc + 1) * 128], ident)
                        if kind == "q":
                            if padk:
                                for half in range(2):
                                    h = 2 * dc + half
                                    sl = slice(64 * half, 64 * (half + 1))
                                    nc.vector.tensor_scalar(
                                        out=qt_sb[sl, h, rb * 128 : (rb + 1) * 128],
                                        in0=tp[sl, :],
                                        scalar1=gamma_t[sl, dc : dc + 1],
                                        scalar2=beta_t[sl, dc : dc + 1],
                                        op0=mybir.AluOpType.mult,
                                        op1=mybir.AluOpType.add,
                                    )
                            else:
                                nc.vector.tensor_scalar(
                                    out=qt_sb[:, dc, rb * 128 : (rb + 1) * 128],
                                    in0=tp,
                                    scalar1=gamma_t[:, dc : dc + 1],
                                    scalar2=beta_t[:, dc : dc + 1],
                                    op0=mybir.AluOpType.mult,
                                    op1=mybir.AluOpType.add,
                                )
                        else:
                            ktblk = kt_pool.tile([128, 128], F32R, tag="ktb", bufs=3)
                            nc.vector.tensor_scalar(
                                out=ktblk,
                                in0=tp,
                                scalar1=gamma_t[:, dc : dc + 1],
                                scalar2=beta_t[:, dc : dc + 1],
                                op0=mybir.AluOpType.mult,
                                op1=mybir.AluOpType.add,
                            )
                            nc.sync.dma_start(
                                out=kt_in_v[dc][:, rb * 128 : (rb + 1) * 128],
                                in_=ktblk,
                            )

            with (
                tc.tile_pool(name="p1_in", bufs=2) as in_pool,
                tc.tile_pool(name="p1_w", bufs=2) as w_pool,
                tc.tile_pool(name="p1_nat", bufs=3) as nat_pool,
                tc.tile_pool(name="p1_kt", bufs=1) as kt_pool,
                tc.tile_pool(name="p1_tmp", bufs=4) as tmp_pool,
                tc.tile_pool(name="ppsum", bufs=2, space="PSUM") as ppsum,
                tc.tile_pool(name="tpsum", bufs=2, space="PSUM") as tpsum,
            ):
                pools = (in_pool, w_pool, nat_pool, kt_pool, tmp_pool, ppsum, tpsum)
                projection(pools, xk, wk, "k")
                if splitag:
                    if mock_collective:
                        for j in range(4):
                            nc.sync.dma_start(
                                out=ktg[:][j * KT_ELEMS : (j + 1) * KT_ELEMS],
                                in_=kv_in[:][0:KT_ELEMS],
                            )
                    else:
                        nc.gpsimd.collective_compute(
                            "AllGather",
                            mybir.AluOpType.bypass,
                            replica_groups=REPLICA_GROUPS,
                            ins=[kv_in[:][0:KT_ELEMS]],
                            outs=[ktg[:]],
                        )
                projection(pools, xv, wv, "v")
                if splitag:
                    if mock_collective:
                        for j in range(4):
                            nc.sync.dma_start(
                                out=vg[:][j * KT_ELEMS : (j + 1) * KT_ELEMS],
                                in_=kv_in[:][KT_ELEMS:KV_ELEMS],
                            )
                    else:
                        nc.gpsimd.collective_compute(
                            "AllGather",
                            mybir.AluOpType.bypass,
                            replica_groups=REPLICA_GROUPS,
                            ins=[kv_in[:][KT_ELEMS:KV_ELEMS]],
                            outs=[vg[:]],
                        )
                elif mock_collective:
                    for j in range(4):
                        nc.sync.dma_start(
                            out=kv_out[:][j * KV_ELEMS : (j + 1) * KV_ELEMS],
                            in_=kv_in[:],
                        )
                else:
                    nc.gpsimd.collective_compute(
                        "AllGather",
                        mybir.AluOpType.bypass,
                        replica_groups=REPLICA_GROUPS,
                        ins=[kv_in[:]],
                        outs=[kv_out[:]],
                    )
                projection(pools, xq, wq, "q")
                nc.sync.dma_start(
                    out=wo_sb, in_=wo[:].rearrange("(c p) f -> p c f", p=128)
                )

            # ---------------- Phase 3: attention ------------------------------
            kv4 = kv_out[:].rearrange("(j e) -> j e", j=4)  # per-peer blocks
            ktg4 = ktg[:].rearrange("(j e) -> j e", j=4)
            vg4 = vg[:].rearrange("(j e) -> j e", j=4)

            with (
                tc.tile_pool(name="a_kt", bufs=2) as kt_pool3,
                tc.tile_pool(name="a_v", bufs=2) as v_pool,
                tc.tile_pool(name="a_exp", bufs=2) as e_pool,
                tc.tile_pool(name="a_den", bufs=2) as den_pool,
                tc.tile_pool(
                    name="s_psum", bufs=(4 if exp1 else 2), space="PSUM"
                ) as s_psum,
                tc.tile_pool(
                    name="c_psum", bufs=(2 if ctx2 else 1), space="PSUM"
                ) as c_psum,
                tc.tile_pool(name="b_psum", bufs=1, space="PSUM") as b_psum,
                tc.tile_pool(
                    name="o_psum", bufs=(1 if ctx2 else 2), space="PSUM"
                ) as o_psum,
                tc.tile_pool(name="out_sb", bufs=2) as out_pool,
            ):
                static_kv = [None, None]
                for hp in range(8):  # head pairs
                    if skip_kv_loads and hp > 0:
                        kt_t, v_t = static_kv
                    else:
                        # K^T for heads (2hp, 2hp+1): [128 dims, 2048 kpos]
                        kt_base = ktg4 if splitag else kv4
                        kt_src = (
                            kt_base[:, hp * (128 * ROWS) : (hp + 1) * (128 * ROWS)]
                            .rearrange("j (p f) -> p j f", p=128)
                        )
                        kt_t = kt_pool3.tile([128, 4, ROWS], F32R, tag="kt")
                        nc.sync.dma_start(out=kt_t, in_=kt_src)
                        # V for the pair: [128 kpos, 16 chunks, 2 heads, 65]
                        v_t = v_pool.tile([128, 16, 2, 65], F32R, tag="v")
                        for j in range(4):
                            for hh in range(2):
                                v_blk = (
                                    vg4[j, :]
                                    if splitag
                                    else kv4[j, KT_ELEMS:KV_ELEMS]
                                )
                                v_src = v_blk.rearrange(
                                    "(r p n) -> p r n", p=128, n=D
                                )[:, :, 128 * hp + 64 * hh : 128 * hp + 64 * (hh + 1)]
                                nc.scalar.dma_start(
                                    out=v_t[:, j * 4 : (j + 1) * 4, hh, 0:64],
                                    in_=v_src,
                                )
                        nc.vector.tensor_copy(
                            v_t[:, :, :, 64:65].rearrange("p c hh one -> p (c hh one)"),
                            ones32,
                        )
                        static_kv = [kt_t, v_t]

                    for hh in range(2):
                        exp_t = e_pool.tile([128, 16, 512], F32R, tag="exp")
                        n_grp, per_grp = (16, 1) if exp1 else (8, 2)
                        for g in range(n_grp):
                            st = s_psum.tile([128, per_grp, 512], F32, tag="st")
                            for k in range(per_grp):
                                cb = per_grp * g + k
                                j, rb = cb // 4, cb % 4
                                if padk:
                                    nc.tensor.matmul(
                                        st[:, k, :],
                                        lhsT=kt_t[:, j, rb * 128 : (rb + 1) * 128],
                                        rhs=qt_sb[:, 2 * hp + hh, :],
                                        start=True,
                                        stop=True,
                                    )
                                else:
                                    nc.tensor.matmul(
                                        st[:, k, :],
                                        lhsT=kt_t[
                                            64 * hh : 64 * (hh + 1),
                                            j,
                                            rb * 128 : (rb + 1) * 128,
                                        ],
                                        rhs=qt_sb[64 * hh : 64 * (hh + 1), hp, :],
                                        start=True,
                                        stop=True,
                                    )
                            nc.scalar.activation(
                                out=exp_t[:, per_grp * g : per_grp * (g + 1), :],
                                in_=st,
                                func=mybir.ActivationFunctionType.Exp,
                                scale=SCALE,
                            )
                        ctx_ps = c_psum.tile([128, 512], F32, tag="ctx")
                        for cb in range(16):
                            nc.tensor.matmul(
                                ctx_ps[0:65, :],
                                lhsT=v_t[:, cb, hh, :],
                                rhs=exp_t[:, cb, :],
                                start=(cb == 0),
                                stop=(cb == 15),
                            )
                        den = den_pool.tile([1, 512], F32R, tag="den")
                        with nc.allow_low_precision(
                            reason="reciprocal rounded to f32r to feed PE broadcast"
                        ):
                            nc.vector.reciprocal(out=den, in_=ctx_ps[64:65, :])
                        bc = b_psum.tile([64, 512], F32, tag="bc")
                        nc.tensor.matmul(
                            bc, lhsT=ones64, rhs=den, start=True, stop=True
                        )
                        bc_sb = den_pool.tile([64, 512], F32, tag="bc_sb")
                        nc.vector.tensor_copy(bc_sb, bc)
                        nc.vector.tensor_mul(
                            ctx_sb[64 * hh : 64 * (hh + 1), hp, :],
                            ctx_ps[0:64, :],
                            bc_sb,
                        )

                # ---------------- Output projection ---------------------------
                for qb in range(4):
                    ot = out_pool.tile([128, D], F32, tag="ot")
                    for ncol in range(2):
                        ps = o_psum.tile([128, 512], F32, tag="po")
                        for dc in range(8):
                            nc.tensor.matmul(
                                ps,
                                lhsT=ctx_sb[:, dc, qb * 128 : (qb + 1) * 128],
                                rhs=wo_sb[:, dc, ncol * 512 : (ncol + 1) * 512],
                                start=(dc == 0),
                                stop=(dc == 7),
                            )
                        nc.vector.tensor_copy(ot[:, ncol * 512 : (ncol + 1) * 512], ps)
                    nc.sync.dma_start(out=out_ext[qb * 128 : (qb + 1) * 128, :], in_=ot)

    nc.finalize()
    return nc


_NC_CACHE = None


def _get_nc():
    global _NC_CACHE
    if _NC_CACHE is None:
        _NC_CACHE = build_kernel(padk=True)
    return _NC_CACHE


def shard_inputs(query, key, value, Wq, Wk, Wv, Wo, ln_gamma, ln_beta):
    q2 = np.ascontiguousarray(query.reshape(B * S, D))
    k2 = np.ascontiguousarray(key.reshape(B * S, D))
    v2 = np.ascontiguousarray(value.reshape(B * S, D))
    wqt = np.ascontiguousarray(Wq.T)
    wkt = np.ascontiguousarray(Wk.T)
    wvt = np.ascontiguousarray(Wv.T)
    wot = np.ascontiguousarray(Wo.T)
    g = np.ascontiguousarray(ln_gamma)
    b = np.ascontiguousarray(ln_beta)
    in_maps = []
    for c in range(N_CORES):
        rows = slice(c * ROWS, (c + 1) * ROWS)
        in_maps.append(
            {
                "xq": np.ascontiguousarray(q2[rows].T),
                "xk": np.ascontiguousarray(k2[rows].T),
                "xv": np.ascontiguousarray(v2[rows].T),
                "wq": wqt,
                "wk": wkt,
                "wv": wvt,
                "wo": wot,
                "gamma": g,
                "beta": b,
            }
        )
    return in_maps


def kernel(query, key, value, Wq, Wk, Wv, Wo, ln_gamma, ln_beta):
    query = np.asarray(query, dtype=np.float32)
    key = np.asarray(key, dtype=np.float32)
    value = np.asarray(value, dtype=np.float32)
    in_maps = shard_inputs(
        query,
        key,
        value,
        np.asarray(Wq, np.float32),
        np.asarray(Wk, np.float32),
        np.asarray(Wv, np.float32),
        np.asarray(Wo, np.float32),
        np.asarray(ln_gamma, np.float32),
        np.asarray(ln_beta, np.float32),
    )
    nc = _get_nc()
    res = run_bass_kernel_spmd(nc, in_maps, core_ids=list(range(N_CORES)))
    out = np.concatenate(
        [res.results[c]["out"] for c in range(N_CORES)], axis=0
    ).reshape(B, S, D)
    return out.astype(np.float32)


# revision 23
# speedup vs baseline: 1.1432x; 1.0806x over previous
"""Distributed multi-head attention (B=2, S=2048, D=1024, H=16) on 8 NeuronCores.

Sharding: tokens are flattened to 4096 rows and split 512 rows/core (core c
owns batch c//4, sequence block c%4).  Each core projects Q/K/V for its own
rows (QK-layernorm is purely local since every core holds full d_model rows),
then K^T and V are AllGathered within each 4-core batch group so every core
holds the full-sequence K/V for its batch.  Attention and the output
projection then run fully locally for the core's 512 query rows (all heads),
so the final output needs no collective — the host just concatenates row
blocks.  All matmuls run in float32r (full-rate PE mode).
"""

import sys

sys.path.insert(0, "/opt/trn_rl_repo")

import numpy as np
import concourse.bass as bass
import concourse.tile as tile
from concourse import bacc, mybir
from concourse.bass_utils import run_bass_kernel_spmd
from concourse.masks import make_identity

F32 = mybir.dt.float32
F32R = mybir.dt.float32r

N_CORES = 8
B, S, D = 2, 2048, 1024
H, DK = 16, 64
ROWS = 512  # token rows per core
LN_EPS = 1e-5
SCALE = 0.125  # 1/sqrt(DK)

KT_ELEMS = D * ROWS  # 524288, K^T block in kv_in
KV_ELEMS = 2 * KT_ELEMS  # K^T + V per core

REPLICA_GROUPS = [[0, 1, 2, 3], [4, 5, 6, 7]]


def build_kernel(repeat=None, mock_collective=False, skip_kv_loads=False, ctx2=False, padk=False, splitag=False, exp1=False, wsplit=False, kt3=False, vstage=False):
    nc = bacc.Bacc("TRN2", target_bir_lowering=False, debug=False, num_devices=N_CORES)

    # Inputs (host pre-transposed): x^T [D, ROWS] slices and W^T [D, D].
    xq = nc.declare_dram_parameter("xq", [D, ROWS], F32R, isOutput=False)
    xk = nc.declare_dram_parameter("xk", [D, ROWS], F32R, isOutput=False)
    xv = nc.declare_dram_parameter("xv", [D, ROWS], F32R, isOutput=False)
    wq = nc.declare_dram_parameter("wq", [D, D], F32R, isOutput=False)
    wk = nc.declare_dram_parameter("wk", [D, D], F32R, isOutput=False)
    wv = nc.declare_dram_parameter("wv", [D, D], F32R, isOutput=False)
    wo = nc.declare_dram_parameter("wo", [D, D], F32R, isOutput=False)
    gamma = nc.declare_dram_parameter("gamma", [D], F32, isOutput=False)
    beta = nc.declare_dram_parameter("beta", [D], F32, isOutput=False)
    out_ext = nc.declare_dram_parameter("out", [ROWS, D], F32, isOutput=True)

    # Collective buffers: [K^T (524288) | V (524288)] flat.
    kv_in = nc.dram_tensor("kv_in", [KV_ELEMS], F32R)
    kv_out = nc.dram_tensor("kv_out", [4 * KV_ELEMS], F32R)
    # split-AG variant: separate gathered buffers for K^T and V
    ktg = nc.dram_tensor("ktg", [4 * KT_ELEMS], F32R)
    vg = nc.dram_tensor("vg", [4 * KT_ELEMS], F32R)

    # DRAM views
    kt_in_v = kv_in[:][0:KT_ELEMS].rearrange("(c p f) -> c p f", p=128, f=ROWS)
    v_in_v = kv_in[:][KT_ELEMS:KV_ELEMS].rearrange("(r p n) -> r p n", p=128, n=D)

    import contextlib

    with tile.TileContext(nc) as tc:
        loop_cm = (
            tc.For_i(
                0,
                repeat,
                1,
                hint_engines=(
                    mybir.EngineType.PE,
                    mybir.EngineType.Activation,
                    mybir.EngineType.DVE,
                    mybir.EngineType.SP,
                    mybir.EngineType.Pool,
                ),
            )
            if repeat
            else contextlib.nullcontext()
        )
        with loop_cm, tc.tile_pool(name="persist", bufs=1) as persist:
            ident = persist.tile([128, 128], F32)
            make_identity(nc, ident)
            eps_t = persist.tile([128, 1], F32)
            nc.vector.memset(eps_t, LN_EPS)
            ones64_f = persist.tile([1, 64], F32)
            nc.vector.memset(ones64_f, 1.0)
            ones64 = persist.tile([1, 64], F32R)
            nc.vector.tensor_copy(ones64, ones64_f)
            ones32 = persist.tile([128, 32], F32)
            nc.vector.memset(ones32, 1.0)
            # gamma/beta per-partition in transposed layout: [128, 8]
            gamma_t = persist.tile([128, 8], F32)
            nc.sync.dma_start(out=gamma_t, in_=gamma[:].rearrange("(c p) -> p c", p=128))
            beta_t = persist.tile([128, 8], F32)
            nc.sync.dma_start(out=beta_t, in_=beta[:].rearrange("(c p) -> p c", p=128))

            if padk:
                # per-head Q^T, zero-padded to full 128 contract rows
                qt_sb = persist.tile([128, 16, ROWS], F32R)
                zero64 = persist.tile([64, ROWS], F32)
                nc.vector.memset(zero64, 0.0)
                for h in range(16):
                    lo = 64 if h % 2 == 0 else 0
                    nc.vector.tensor_copy(qt_sb[lo : lo + 64, h, :], zero64)
            else:
                qt_sb = persist.tile([128, 8, ROWS], F32R)  # Q^T (post-LN)
            ctx_sb = persist.tile([128, 8, ROWS], F32R)  # context^T
            wo_sb = persist.tile([128, 8, D], F32R)  # Wo^T

            # ---------------- Phase 1: projections + LN + transposes ----------
            def projection(ctx_pools, xin, win, kind):
                """kind: 'q' | 'k' | 'v'."""
                in_pool, w_pool, nat_pool, kt_pool, tmp_pool, ppsum, tpsum = ctx_pools
                xt = in_pool.tile([128, 8, ROWS], F32R, tag="xt")
                xin_v = xin[:].rearrange("(c p) f -> p c f", p=128)
                w_sb = w_pool.tile([128, 8, D], F32R, tag="w")
                win_v = win[:].rearrange("(c p) f -> p c f", p=128)
                w_eng = nc.scalar if wsplit else nc.sync
                for dc in range(8):
                    nc.sync.dma_start(out=xt[:, dc, :], in_=xin_v[:, dc, :])
                    w_eng.dma_start(out=w_sb[:, dc, :], in_=win_v[:, dc, :])



                for rb in range(4):
                    nat = nat_pool.tile([128, D], F32 if kind != "v" else F32R, tag="nat")
                    for ncol in range(2):
                        ps = ppsum.tile([128, 512], F32, tag="proj")
                        for dc in range(8):
                            nc.tensor.matmul(
                                ps,
                                lhsT=xt[:, dc, rb * 128 : (rb + 1) * 128],
                                rhs=w_sb[:, dc, ncol * 512 : (ncol + 1) * 512],
                                start=(dc == 0),
                                stop=(dc == 7),
                            )
                        nc.vector.tensor_copy(nat[:, ncol * 512 : (ncol + 1) * 512], ps)
                    if kind == "v":
                        nc.sync.dma_start(out=v_in_v[rb], in_=nat)
                        continue
                    # LayerNorm over the full row (free dim), gamma/beta applied
                    # post-transpose (where they are per-partition).
                    stats = tmp_pool.tile([128, 2, 6], F32, tag="stats")
                    for i in range(2):
                        nc.vector.bn_stats(
                            out=stats[:, i, :], in_=nat[:, i * 512 : (i + 1) * 512]
                        )
                    mv = tmp_pool.tile([128, 2], F32, tag="mv")
                    nc.vector.bn_aggr(out=mv, in_=stats)
                    rstd = tmp_pool.tile([128, 1], F32, tag="rstd")
                    nc.scalar.activation(
                        out=rstd,
                        in_=mv[:, 1:2],
                        func=mybir.ActivationFunctionType.Sqrt,
                        bias=eps_t,
                        scale=1.0,
                    )
                    nc.vector.reciprocal(out=rstd, in_=rstd)
                    nc.vector.tensor_scalar(
                        out=nat,
                        in0=nat,
                        scalar1=mv[:, 0:1],
                        scalar2=rstd,
                        op0=mybir.AluOpType.subtract,
                        op1=mybir.AluOpType.mult,
                    )
                    # transpose 128x128 blocks into [D, ROWS] layout
                    for dc in range(8):
                        tp = tpsum.tile([128, 128], F32, tag="tp")
                        nc.tensor.transpose(tp, nat[:, dc * 128 : (dc + 1) * 128], ident)
                        if kind == "q":
                            if padk:
                                for half in range(2):
                                    h = 2 * dc + half
                                    sl = slice(64 * half, 64 * (half + 1))
                                    nc.vector.tensor_scalar(
                                        out=qt_sb[sl, h, rb * 128 : (rb + 1) * 128],
                                        in0=tp[sl, :],
                                        scalar1=gamma_t[sl, dc : dc + 1],
                                        scalar2=beta_t[sl, dc : dc + 1],
                                        op0=mybir.AluOpType.mult,
                                        op1=mybir.AluOpType.add,
                                    )
                            else:
                                nc.vector.tensor_scalar(
                                    out=qt_sb[:, dc, rb * 128 : (rb + 1) * 128],
                                    in0=tp,
                                    scalar1=gamma_t[:, dc : dc + 1],
                                    scalar2=beta_t[:, dc : dc + 1],
                                    op0=mybir.AluOpType.mult,
                                    op1=mybir.AluOpType.add,
                                )
                        else:
                            ktblk = kt_pool.tile([128, 128], F32R, tag="ktb", bufs=3)
                            nc.vector.tensor_scalar(
                                out=ktblk,
                                in0=tp,
                                scalar1=gamma_t[:, dc : dc + 1],
                                scalar2=beta_t[:, dc : dc + 1],
                                op0=mybir.AluOpType.mult,
                                op1=mybir.AluOpType.add,
                            )
                            nc.sync.dma_start(
                                out=kt_in_v[dc][:, rb * 128 : (rb + 1) * 128],
                                in_=ktblk,
                            )

            with (
                tc.tile_pool(name="p1_in", bufs=2) as in_pool,
                tc.tile_pool(name="p1_w", bufs=2) as w_pool,
                tc.tile_pool(name="p1_nat", bufs=3) as nat_pool,
                tc.tile_pool(name="p1_kt", bufs=1) as kt_pool,
                tc.tile_pool(name="p1_tmp", bufs=4) as tmp_pool,
                tc.tile_pool(name="ppsum", bufs=2, space="PSUM") as ppsum,
                tc.tile_pool(name="tpsum", bufs=2, space="PSUM") as tpsum,
            ):
                pools = (in_pool, w_pool, nat_pool, kt_pool, tmp_pool, ppsum, tpsum)
                projection(pools, xk, wk, "k")
                if splitag:
                    if mock_collective:
                        for j in range(4):
                            nc.sync.dma_start(
                                out=ktg[:][j * KT_ELEMS : (j + 1) * KT_ELEMS],
                                in_=kv_in[:][0:KT_ELEMS],
                            )
                    else:
                        nc.gpsimd.collective_compute(
                            "AllGather",
                            mybir.AluOpType.bypass,
                            replica_groups=REPLICA_GROUPS,
                            ins=[kv_in[:][0:KT_ELEMS]],
                            outs=[ktg[:]],
                        )
                projection(pools, xv, wv, "v")
                if splitag:
                    if mock_collective:
                        for j in range(4):
                            nc.sync.dma_start(
                                out=vg[:][j * KT_ELEMS : (j + 1) * KT_ELEMS],
                                in_=kv_in[:][KT_ELEMS:KV_ELEMS],
                            )
                    else:
                        nc.gpsimd.collective_compute(
                            "AllGather",
                            mybir.AluOpType.bypass,
                            replica_groups=REPLICA_GROUPS,
                            ins=[kv_in[:][KT_ELEMS:KV_ELEMS]],
                            outs=[vg[:]],
                        )
                elif mock_collective:
                    for j in range(4):
                        nc.sync.dma_start(
                            out=kv_out[:][j * KV_ELEMS : (j + 1) * KV_ELEMS],
                            in_=kv_in[:],
                        )
                else:
                    nc.gpsimd.collective_compute(
                        "AllGather",
                        mybir.AluOpType.bypass,
                        replica_groups=REPLICA_GROUPS,
                        ins=[kv_in[:]],
                        outs=[kv_out[:]],
                    )
                projection(pools, xq, wq, "q")
                (nc.scalar if wsplit else nc.sync).dma_start(
                    out=wo_sb, in_=wo[:].rearrange("(c p) f -> p c f", p=128)
                )

            # ---------------- Phase 3: attention ------------------------------
            kv4 = kv_out[:].rearrange("(j e) -> j e", j=4)  # per-peer blocks
            ktg4 = ktg[:].rearrange("(j e) -> j e", j=4)
            vg4 = vg[:].rearrange("(j e) -> j e", j=4)

            with (
                tc.tile_pool(name="a_kt", bufs=(3 if kt3 else 2)) as kt_pool3,
                tc.tile_pool(name="a_v", bufs=2) as v_pool,
                tc.tile_pool(name="a_exp", bufs=2) as e_pool,
                tc.tile_pool(name="a_den", bufs=(1 if vstage else 2)) as den_pool,
                tc.tile_pool(
                    name="s_psum", bufs=(4 if exp1 else 2), space="PSUM"
                ) as s_psum,
                tc.tile_pool(
                    name="c_psum", bufs=(2 if ctx2 else 1), space="PSUM"
                ) as c_psum,
                tc.tile_pool(name="b_psum", bufs=1, space="PSUM") as b_psum,
                tc.tile_pool(
                    name="o_psum", bufs=(1 if ctx2 else 2), space="PSUM"
                ) as o_psum,
                tc.tile_pool(name="out_sb", bufs=(1 if vstage else 2)) as out_pool,
            ):
                static_kv = [None, None]
                for hp in range(8):  # head pairs
                    if skip_kv_loads and hp > 0:
                        kt_t, v_t = static_kv
                    else:
                        # K^T for heads (2hp, 2hp+1): [128 dims, 2048 kpos]
                        kt_base = ktg4 if splitag else kv4
                        kt_src = (
                            kt_base[:, hp * (128 * ROWS) : (hp + 1) * (128 * ROWS)]
                            .rearrange("j (p f) -> p j f", p=128)
                        )
                        kt_t = kt_pool3.tile([128, 4, ROWS], F32R, tag="kt")
                        nc.sync.dma_start(out=kt_t, in_=kt_src)
                        # V for the pair: [128 kpos, 16 chunks, 2 heads, 65]
                        v_t = v_pool.tile([128, 16, 2, 65], F32R, tag="v")
                        for j in range(4):
                            v_blk = (
                                vg4[j, :]
                                if splitag
                                else kv4[j, KT_ELEMS:KV_ELEMS]
                            )
                            v_rpn = v_blk.rearrange("(r p n) -> p r n", p=128, n=D)
                            if vstage:
                                vl = v_pool.tile(
                                    [128, 4, 128], F32R, tag="vload", bufs=3
                                )
                                nc.scalar.dma_start(
                                    out=vl,
                                    in_=v_rpn[:, :, 128 * hp : 128 * (hp + 1)],
                                )
                                nc.vector.tensor_copy(
                                    v_t[:, j * 4 : (j + 1) * 4, :, 0:64],
                                    vl.rearrange("p r (hh d) -> p r hh d", d=64),
                                )
                            else:
                                for hh in range(2):
                                    v_src = v_rpn[
                                        :,
                                        :,
                                        128 * hp + 64 * hh : 128 * hp
                                        + 64 * (hh + 1),
                                    ]
                                    nc.scalar.dma_start(
                                        out=v_t[:, j * 4 : (j + 1) * 4, hh, 0:64],
                                        in_=v_src,
                                    )
                        nc.vector.tensor_copy(
                            v_t[:, :, :, 64:65].rearrange("p c hh one -> p (c hh one)"),
                            ones32,
                        )
                        static_kv = [kt_t, v_t]

                    for hh in range(2):
                        exp_t = e_pool.tile([128, 16, 512], F32R, tag="exp")
                        n_grp, per_grp = (16, 1) if exp1 else (8, 2)
                        for g in range(n_grp):
                            st = s_psum.tile([128, per_grp, 512], F32, tag="st")
                            for k in range(per_grp):
                                cb = per_grp * g + k
                                j, rb = cb // 4, cb % 4
                                if padk:
                                    nc.tensor.matmul(
                                        st[:, k, :],
                                        lhsT=kt_t[:, j, rb * 128 : (rb + 1) * 128],
                                        rhs=qt_sb[:, 2 * hp + hh, :],
                                        start=True,
                                        stop=True,
                                    )
                                else:
                                    nc.tensor.matmul(
                                        st[:, k, :],
                                        lhsT=kt_t[
                                            64 * hh : 64 * (hh + 1),
                                            j,
                                            rb * 128 : (rb + 1) * 128,
                                        ],
                                        rhs=qt_sb[64 * hh : 64 * (hh + 1), hp, :],
                                        start=True,
                                        stop=True,
                                    )
                            nc.scalar.activation(
                                out=exp_t[:, per_grp * g : per_grp * (g + 1), :],
                                in_=st,
                                func=mybir.ActivationFunctionType.Exp,
                                scale=SCALE,
                            )
                        ctx_ps = c_psum.tile([128, 512], F32, tag="ctx")
                        for cb in range(16):
                            nc.tensor.matmul(
                                ctx_ps[0:65, :],
                                lhsT=v_t[:, cb, hh, :],
                                rhs=exp_t[:, cb, :],
                                start=(cb == 0),
                                stop=(cb == 15),
                            )
                        den = den_pool.tile([1, 512], F32R, tag="den")
                        with nc.allow_low_precision(
                            reason="reciprocal rounded to f32r to feed PE broadcast"
                        ):
                            nc.vector.reciprocal(out=den, in_=ctx_ps[64:65, :])
                        bc = b_psum.tile([64, 512], F32, tag="bc")
                        nc.tensor.matmul(
                            bc, lhsT=ones64, rhs=den, start=True, stop=True
                        )
                        bc_sb = den_pool.tile([64, 512], F32, tag="bc_sb")
                        nc.vector.tensor_copy(bc_sb, bc)
                        nc.vector.tensor_mul(
                            ctx_sb[64 * hh : 64 * (hh + 1), hp, :],
                            ctx_ps[0:64, :],
                            bc_sb,
                        )

                # ---------------- Output projection ---------------------------
                for qb in range(4):
                    ot = out_pool.tile([128, D], F32, tag="ot")
                    for ncol in range(2):
                        ps = o_psum.tile([128, 512], F32, tag="po")
                        for dc in range(8):
                            nc.tensor.matmul(
                                ps,
                                lhsT=ctx_sb[:, dc, qb * 128 : (qb + 1) * 128],
                                rhs=wo_sb[:, dc, ncol * 512 : (ncol + 1) * 512],
                                start=(dc == 0),
                                stop=(dc == 7),
                            )
                        nc.vector.tensor_copy(ot[:, ncol * 512 : (ncol + 1) * 512], ps)
                    nc.sync.dma_start(out=out_ext[qb * 128 : (qb + 1) * 128, :], in_=ot)

    nc.finalize()
    return nc


_NC_CACHE = None


def _get_nc():
    global _NC_CACHE
    if _NC_CACHE is None:
        _NC_CACHE = build_kernel(padk=True, wsplit=True, kt3=True)
    return _NC_CACHE


def shard_inputs(query, key, value, Wq, Wk, Wv, Wo, ln_gamma, ln_beta):
    q2 = np.ascontiguousarray(query.reshape(B * S, D))
    k2 = np.ascontiguousarray(key.reshape(B * S, D))
    v2 = np.ascontiguousarray(value.reshape(B * S, D))
    wqt = np.ascontiguousarray(Wq.T)
    wkt = np.ascontiguousarray(Wk.T)
    wvt = np.ascontiguousarray(Wv.T)
    wot = np.ascontiguousarray(Wo.T)
    g = np.ascontiguousarray(ln_gamma)
    b = np.ascontiguousarray(ln_beta)
    in_maps = []
    for c in range(N_CORES):
        rows = slice(c * ROWS, (c + 1) * ROWS)
        in_maps.append(
            {
                "xq": np.ascontiguousarray(q2[rows].T),
                "xk": np.ascontiguousarray(k2[rows].T),
                "xv": np.ascontiguousarray(v2[rows].T),
                "wq": wqt,
                "wk": wkt,
                "wv": wvt,
                "wo": wot,
                "gamma": g,
                "beta": b,
            }
        )
    return in_maps


def kernel(query, key, value, Wq, Wk, Wv, Wo, ln_gamma, ln_beta):
    query = np.asarray(query, dtype=np.float32)
    key = np.asarray(key, dtype=np.float32)
    value = np.asarray(value, dtype=np.float32)
    in_maps = shard_inputs(
        query,
        key,
        value,
        np.asarray(Wq, np.float32),
        np.asarray(Wk, np.float32),
        np.asarray(Wv, np.float32),
        np.asarray(Wo, np.float32),
        np.asarray(ln_gamma, np.float32),
        np.asarray(ln_beta, np.float32),
    )
    nc = _get_nc()
    res = run_bass_kernel_spmd(nc, in_maps, core_ids=list(range(N_CORES)))
    out = np.concatenate(
        [res.results[c]["out"] for c in range(N_CORES)], axis=0
    ).reshape(B, S, D)
    return out.astype(np.float32)
